# revision 5
# baseline (speedup 1.0000x reference)
"""Trainium2 Bass kernel for nn_OmegaEntangle (E^T C E with entangle coefficients).

Math (validated vs reference to ~8e-7 rel err in fp32):
  p_i = sum_j v_ij^2 ; m_i = mean_j v_ij
  C[i,j] = mask(i<j) * sqrt(p_i p_j) * (m_i + 1j*m_j) / sqrt(m_i^2 + m_j^2)
  out = E^T C E   (complex, E real)  ->  out_re = E^T Cr E, out_im = E^T Ci E

Sharding: data-parallel over the 2048 OUTPUT COLUMNS (256 per core).
Each core:
  - reduces its 64-row shard of vulns to p/m partials (ACT square+accum, DVE accum)
  - AllGather of tiny derived vectors (sp, a=sp*m, m2=m^2) across 8 cores
  - builds the full C^T (real+imag) [512,512] from the gathered vectors
  - computes T = C @ E[:, cols]  (PE),  out[:, cols] = E^T @ T  (PE)
  - writes its [2048, 256] slab of out_re / out_im
Host concatenates slabs along columns -> [2048, 2048] complex64.
"""

import numpy as np

import concourse.bass as bass
import concourse.mybir as mybir
import concourse.tile as tile
from concourse import bacc
from concourse.bass_utils import run_bass_kernel_spmd

D = 512          # number of domains
V = 32768        # vuln dim
S = 2048         # sup (embed) dim
NCORES = 8
ROWS_PER_CORE = D // NCORES          # 64
COLS_PER_CORE = S // NCORES          # 256
NVT = 8                               # number of vuln tiles per core
VFREE = (ROWS_PER_CORE * V) // (128 * NVT)   # 2048 free elems per vuln tile
KT = D // 128                         # 4 contraction tiles
MT = S // 128                         # 16 output row tiles
INV_V = 1.0 / V

F32 = mybir.dt.float32
AF = mybir.ActivationFunctionType
ALU = mybir.AluOpType

_CACHE = {}


def build_bass():
    nc = bacc.Bacc(
        "TRN2", target_bir_lowering=False, debug=False, num_devices=NCORES
    )

    v128 = nc.dram_tensor("v128", [128, NVT, VFREE], F32, kind="ExternalInput")
    efull = nc.dram_tensor("efull", [KT, 128, S], F32, kind="ExternalInput")
    ecols = nc.dram_tensor("ecols", [KT, 128, COLS_PER_CORE], F32, kind="ExternalInput")
    pairmat = nc.dram_tensor("pairmat", [128, ROWS_PER_CORE], F32, kind="ExternalInput")
    out_re = nc.dram_tensor("out_re", [S, COLS_PER_CORE], F32, kind="ExternalOutput")
    out_im = nc.dram_tensor("out_im", [S, COLS_PER_CORE], F32, kind="ExternalOutput")

    with tile.TileContext(nc) as tc:
        with (
            tc.tile_pool(name="vin", bufs=3) as vin_pool,
            tc.tile_pool(name="scr", bufs=2) as scr_pool,
            tc.tile_pool(name="epool", bufs=1) as e_pool,
            tc.tile_pool(name="small", bufs=1) as small_pool,
            tc.tile_pool(name="cbuild", bufs=2) as cb_pool,
            tc.tile_pool(name="ctp", bufs=1) as ct_pool,
            tc.tile_pool(name="tsb", bufs=1) as t_pool,
            tc.tile_pool(name="psA", bufs=4, space="PSUM") as psA,
            tc.tile_pool(name="psB", bufs=4, space="PSUM") as psB,
            tc.tile_pool(name="dram", bufs=1, space="DRAM") as dram_pool,
        ):
            # ---------------- Phase 0: issue vuln DMAs first, then E ----------
            vts = []
            for t in range(NVT):
                vt = vin_pool.tile([128, VFREE], F32, name=f"vt{t}", tag="vt")
                nc.sync.dma_start(vt[:], v128[:, t, :])
                vts.append(vt)

            e_sb = []
            for kt in range(KT):
                et = e_pool.tile([128, S], F32, name=f"e{kt}", tag=f"e{kt}")
                nc.sync.dma_start(et[:], efull[kt])
                e_sb.append(et)
            ec_sb = []
            for kt in range(KT):
                ect = e_pool.tile(
                    [128, COLS_PER_CORE], F32, name=f"ec{kt}", tag=f"ec{kt}"
                )
                nc.sync.dma_start(ect[:], ecols[kt])
                ec_sb.append(ect)
            pair_sb = small_pool.tile([128, ROWS_PER_CORE], F32, name="pair_sb")
            nc.sync.dma_start(pair_sb[:], pairmat[:])

            # ---------------- Phase 1: p/m reduction of the row shard ---------
            # pm_acc columns: [0:NVT] = per-tile sum(x^2); [NVT:2*NVT] = sum(x)
            pm_acc = small_pool.tile([128, 2 * NVT], F32, name="pm_acc")
            for t in range(NVT):
                sq = scr_pool.tile([128, VFREE], F32, name="sq", tag="sq")
                nc.scalar.activation(
                    sq[:], vts[t][:], AF.Square,
                    accum_out=pm_acc[:, t : t + 1],
                )
                raw = scr_pool.tile([128, VFREE], F32, name="raw", tag="raw")
                nc.vector.tensor_scalar(
                    raw[:], vts[t][:], 1.0, None, ALU.mult, ALU.add,
                    accum_out=pm_acc[:, NVT + t : NVT + t + 1],
                )

            # pair-combine [128] -> [64] rows via matmul with pair matrix
            ps_pm = psB.tile([ROWS_PER_CORE, 2 * NVT], F32, name="ps_pm", tag="o")
            nc.tensor.matmul(ps_pm[:], pair_sb[:], pm_acc[:], start=True, stop=True)

            p_sh = small_pool.tile([ROWS_PER_CORE, 1], F32, name="p_sh")
            msum_sh = small_pool.tile([ROWS_PER_CORE, 1], F32, name="msum_sh")
            nc.vector.tensor_reduce(p_sh[:], ps_pm[:, 0:NVT], mybir.AxisListType.X, ALU.add)
            nc.vector.tensor_reduce(
                msum_sh[:], ps_pm[:, NVT : 2 * NVT], mybir.AxisListType.X, ALU.add
            )

            # derived shard vectors, packed into Dv = [sp | a | m2]  [64, 3]
            Dv = small_pool.tile([ROWS_PER_CORE, 3], F32, name="Dv")
            nc.scalar.activation(Dv[:, 0:1], p_sh[:], AF.Sqrt)  # sp = sqrt(p)
            # a = sp * (msum/V)
            nc.vector.scalar_tensor_tensor(
                Dv[:, 1:2], msum_sh[:], INV_V, Dv[:, 0:1], op0=ALU.mult, op1=ALU.mult
            )
            # m2 = (msum/V)^2 = msum^2 * (1/V^2)
            nc.vector.scalar_tensor_tensor(
                Dv[:, 2:3], msum_sh[:], INV_V * INV_V, msum_sh[:],
                op0=ALU.mult, op1=ALU.mult,
            )

            # ---------------- Phase 2: AllGather of derived vectors -----------
            cc_in = dram_pool.tile([3 * ROWS_PER_CORE], F32, name="cc_in")
            cc_out = dram_pool.tile(
                [NCORES, 3 * ROWS_PER_CORE], F32, name="cc_out", addr_space="Shared"
            )
            # Dv [64p, 3f] -> dram layout [3][64] (vector-major)
            nc.sync.dma_start(cc_in.rearrange("(j q) -> q j", q=ROWS_PER_CORE), Dv[:])
            nc.gpsimd.collective_compute(
                "AllGather",
                ALU.bypass,
                replica_groups=[list(range(NCORES))],
                ins=[cc_in[:]],
                outs=[cc_out[:]],
            )

            # row-layout loads: full vectors as [1, 512] (i = 64*r + q)
            sp_row = small_pool.tile([1, D], F32, name="sp_row")
            a_row = small_pool.tile([1, D], F32, name="a_row")
            m2_row = small_pool.tile([1, D], F32, name="m2_row")
            cc3 = cc_out.rearrange("r (j q) -> j r q", j=3)  # [3, 8, 64]
            nc.sync.dma_start(sp_row.rearrange("o (r q) -> o r q", r=NCORES), cc3[0])
            nc.sync.dma_start(a_row.rearrange("o (r q) -> o r q", r=NCORES), cc3[1])
            nc.sync.dma_start(m2_row.rearrange("o (r q) -> o r q", r=NCORES), cc3[2])

            # per-partition loads per contraction tile: [128, 1] each
            sp_c, a_c, m2_c = [], [], []
            cc4 = cc_out.rearrange(
                "(t h) (j q) -> t j h q", t=KT, j=3
            )  # [4 jt, 3, 2, 64]
            for jt in range(KT):
                spc = small_pool.tile([128, 1], F32, name=f"spc{jt}", tag=f"spc{jt}")
                ac = small_pool.tile([128, 1], F32, name=f"ac{jt}", tag=f"ac{jt}")
                m2c = small_pool.tile([128, 1], F32, name=f"m2c{jt}", tag=f"m2c{jt}")
                nc.sync.dma_start(spc[:], cc4[jt, 0])
                nc.sync.dma_start(ac[:], cc4[jt, 1])
                nc.sync.dma_start(m2c[:], cc4[jt, 2])
                sp_c.append(spc)
                a_c.append(ac)
                m2_c.append(m2c)

            # broadcast rows to all 128 partitions via K=1 matmuls
            ones_sb = small_pool.tile([1, 128], F32, name="ones_sb")
            nc.vector.memset(ones_sb[:], 1.0)
            sp_bc = small_pool.tile([128, D], F32, name="sp_bc")
            a_bc = small_pool.tile([128, D], F32, name="a_bc")
            m2_bc = small_pool.tile([128, D], F32, name="m2_bc")
            for row, bc in ((sp_row, sp_bc), (a_row, a_bc), (m2_row, m2_bc)):
                ps_bc = psB.tile([128, D], F32, name="ps_bc", tag="o")
                nc.tensor.matmul(ps_bc[:], ones_sb[:], row[:], start=True, stop=True)
                nc.scalar.copy(bc[:], ps_bc[:])

            # ---------------- Phase 3: build C^T (real and imag) --------------
            ct_r, ct_i = [], []
            for jt in range(KT):
                h = cb_pool.tile([128, D], F32, name="h", tag="h")
                nc.scalar.activation(
                    h[:], m2_bc[:], AF.Sqrt, bias=m2_c[jt][:], scale=1.0
                )
                rinv = cb_pool.tile([128, D], F32, name="rinv", tag="rinv")
                nc.vector.reciprocal_approx_fast(out=rinv[:], in_=h[:])
                # strictly-lower mask in (j, i): keep where j_glob - i > 0
                rm = cb_pool.tile([128, D], F32, name="rm", tag="rm")
                nc.gpsimd.affine_select(
                    out=rm[:], in_=rinv[:],
                    pattern=[[-1, D]], compare_op=ALU.is_gt,
                    fill=0.0, base=128 * jt, channel_multiplier=1,
                )
                ctr = ct_pool.tile([128, D], F32, name=f"ctr{jt}", tag=f"ctr{jt}")
                cti = ct_pool.tile([128, D], F32, name=f"cti{jt}", tag=f"cti{jt}")
                # C_r^T[j,i] = a_i * sp_j * rm ;  C_i^T[j,i] = sp_i * a_j * rm
                nc.vector.scalar_tensor_tensor(
                    ctr[:], a_bc[:], sp_c[jt][:], rm[:], op0=ALU.mult, op1=ALU.mult
                )
                nc.vector.scalar_tensor_tensor(
                    cti[:], sp_bc[:], a_c[jt][:], rm[:], op0=ALU.mult, op1=ALU.mult
                )
                ct_r.append(ctr)
                ct_i.append(cti)

            # ---------------- Phase 4: T = C @ E[:, cols]  --------------------
            # psum tile [128, 512]: cols 0:256 = T_r, 256:512 = T_i
            t_sb = []
            for it in range(KT):
                ps_t = psA.tile([128, 2 * COLS_PER_CORE], F32, name="ps_t", tag="t")
                for jt in range(KT):
                    nc.tensor.matmul(
                        ps_t[:, 0:COLS_PER_CORE],
                        ct_r[jt][:, it * 128 : (it + 1) * 128],
                        ec_sb[jt][:],
                        start=(jt == 0), stop=(jt == KT - 1),
                    )
                for jt in range(KT):
                    nc.tensor.matmul(
                        ps_t[:, COLS_PER_CORE : 2 * COLS_PER_CORE],
                        ct_i[jt][:, it * 128 : (it + 1) * 128],
                        ec_sb[jt][:],
                        start=(jt == 0), stop=(jt == KT - 1),
                    )
                tsb = t_pool.tile(
                    [128, 2 * COLS_PER_CORE], F32, name=f"tsb{it}", tag=f"tsb{it}"
                )
                if it % 2 == 0:
                    nc.scalar.copy(tsb[:], ps_t[:])
                else:
                    nc.vector.tensor_copy(tsb[:], ps_t[:])
                t_sb.append(tsb)

            # ---------------- Phase 5: out[:, cols] = E^T @ T -----------------
            for st in range(MT):
                ps_o = psB.tile([128, 2 * COLS_PER_CORE], F32, name="ps_o", tag="o")
                for it in range(KT):
                    nc.tensor.matmul(
                        ps_o[:],
                        e_sb[it][:, st * 128 : (st + 1) * 128],
                        t_sb[it][:],
                        start=(it == 0), stop=(it == KT - 1),
                    )
                osb = scr_pool.tile(
                    [128, 2 * COLS_PER_CORE], F32, name="osb", tag="osb", bufs=4
                )
                if st % 2 == 0:
                    nc.scalar.copy(osb[:], ps_o[:])
                else:
                    nc.vector.tensor_copy(osb[:], ps_o[:])
                nc.sync.dma_start(
                    out_re[st * 128 : (st + 1) * 128, :], osb[:, 0:COLS_PER_CORE]
                )
                nc.sync.dma_start(
                    out_im[st * 128 : (st + 1) * 128, :],
                    osb[:, COLS_PER_CORE : 2 * COLS_PER_CORE],
                )

    nc.compile()
    return nc


def _prepare_in_maps(vulns, embed_table, domain_ids):
    vulns = np.ascontiguousarray(np.asarray(vulns, dtype=np.float32))
    embed_table = np.ascontiguousarray(np.asarray(embed_table, dtype=np.float32))
    domain_ids = np.asarray(domain_ids).astype(np.int64)
    E = np.ascontiguousarray(embed_table[domain_ids])  # [512, 2048]
    e4 = E.reshape(KT, 128, S)
    pair = np.repeat(np.eye(ROWS_PER_CORE, dtype=np.float32), 2, axis=0)
    pair = np.ascontiguousarray(pair)  # [128, 64]
    in_maps = []
    for c in range(NCORES):
        vsh = vulns[c * ROWS_PER_CORE : (c + 1) * ROWS_PER_CORE]
        in_maps.append(
            {
                "v128": np.ascontiguousarray(vsh.reshape(128, NVT, VFREE)),
                "efull": e4,
                "ecols": np.ascontiguousarray(
                    e4[:, :, c * COLS_PER_CORE : (c + 1) * COLS_PER_CORE]
                ),
                "pairmat": pair,
            }
        )
    return in_maps


def kernel(vulns, embed_table, domain_ids, _trace=False):
    if "nc" not in _CACHE:
        _CACHE["nc"] = build_bass()
    nc = _CACHE["nc"]
    in_maps = _prepare_in_maps(vulns, embed_table, domain_ids)
    res = run_bass_kernel_spmd(
        nc, in_maps, core_ids=list(range(NCORES)), trace=_trace
    )
    _CACHE["last_results"] = res
    out = np.empty((S, S), dtype=np.complex64)
    for c in range(NCORES):
        r = res.results[c]
        sl = slice(c * COLS_PER_CORE, (c + 1) * COLS_PER_CORE)
        out[:, sl] = r["out_re"] + 1j * r["out_im"]
    return out


if __name__ == "__main__":
    rng = np.random.default_rng(0)
    v = rng.standard_normal((D, V), dtype=np.float32)
    et = rng.standard_normal((D, S), dtype=np.float32)
    ids = np.arange(D, dtype=np.int32)
    out = kernel(v, et, ids)
    print(out.shape, out.dtype)


# revision 9
# speedup vs baseline: 1.9524x; 1.9524x over previous
"""Trainium2 Bass kernel for nn_OmegaEntangle (E^T C E with entangle coefficients).

Math (validated vs reference to ~8e-7 rel err in fp32):
  p_i = sum_j v_ij^2 ; m_i = mean_j v_ij
  C[i,j] = mask(i<j) * sqrt(p_i p_j) * (m_i + 1j*m_j) / sqrt(m_i^2 + m_j^2)
  out = E^T C E   (complex, E real)  ->  out_re = E^T Cr E, out_im = E^T Ci E

Sharding: data-parallel over the 2048 OUTPUT COLUMNS (256 per core), with the
p/m reduction row-sharded (64 rows per core).

Two NEFF launches (a device collective would cost ~60+ us of entry-barrier +
AllGather latency on this platform for 768 bytes; host concat of the tiny
reduction result is far cheaper):
  Kernel A: each core reduces its [64, 32768] vuln shard -> p[64], msum[64].
  Host: concatenates the 8 shards (pure data movement, no math).
  Kernel B: each core derives sp/a/m2 vectors, builds C^T, computes
    T = C @ E[:, cols] and out[:, cols] = E^T @ T, writes [2048, 256] slabs.
Host concatenates slabs along columns -> [2048, 2048] complex64.
"""

import numpy as np

import concourse.bass as bass
import concourse.mybir as mybir
import concourse.tile as tile
from concourse import bacc
from concourse.bass_utils import run_bass_kernel_spmd

D = 512          # number of domains
V = 32768        # vuln dim
S = 2048         # sup (embed) dim
NCORES = 8
ROWS_PER_CORE = D // NCORES          # 64
COLS_PER_CORE = S // NCORES          # 256
NVT = 8                               # number of vuln tiles per core
VFREE = (ROWS_PER_CORE * V) // (128 * NVT)   # 2048 free elems per vuln tile
KT = D // 128                         # 4 contraction tiles
MT = S // 128                         # 16 output row tiles
INV_V = 1.0 / V
WARMUP_MMS = 14                       # PE warm-up matmuls at kernel-B start

F32 = mybir.dt.float32
F32R = mybir.dt.float32r
BF16 = mybir.dt.bfloat16
# float32r (TF32) matmul inputs stream at 1 cyc/row vs 4 for float32.
# Host pre-rounds E to TF32 values; on-device producers of matmul operands
# write float32r-typed tiles so the BIR verifier sees rounded inputs.


def _tf32_round(x):
    xi = np.ascontiguousarray(x, dtype=np.float32).view(np.uint32)
    return ((xi + np.uint32(0x1000)) & np.uint32(0xFFFFE000)).view(np.float32)
AF = mybir.ActivationFunctionType
ALU = mybir.AluOpType

_CACHE = {}


def build_kernel_a():
    """Reduce kernel: per-core p/msum over the 64-row vuln shard."""
    nc = bacc.Bacc("TRN2", target_bir_lowering=False, debug=False, num_devices=NCORES)

    v128 = nc.dram_tensor("v128", [128, NVT, VFREE], F32, kind="ExternalInput")
    pairmat = nc.dram_tensor("pairmat", [128, ROWS_PER_CORE], F32, kind="ExternalInput")
    out_pm = nc.dram_tensor("out_pm", [ROWS_PER_CORE, 2], F32, kind="ExternalOutput")

    with tile.TileContext(nc) as tc:
        with (
            tc.tile_pool(name="vin", bufs=3) as vin_pool,
            tc.tile_pool(name="scr", bufs=2) as scr_pool,
            tc.tile_pool(name="small", bufs=1) as small_pool,
            tc.tile_pool(name="ps", bufs=1, space="PSUM") as ps_pool,
        ):
            vts = []
            for t in range(NVT):
                vt = vin_pool.tile([128, VFREE], F32, name=f"vt{t}", tag="vt")
                nc.sync.dma_start(vt[:], v128[:, t, :])
                vts.append(vt)
            pair_sb = small_pool.tile([128, ROWS_PER_CORE], F32, name="pair_sb")
            nc.sync.dma_start(pair_sb[:], pairmat[:])

            pm_acc = small_pool.tile([128, 2 * NVT], F32, name="pm_acc")
            for t in range(NVT):
                sq = scr_pool.tile([128, VFREE], F32, name="sq", tag="sq")
                nc.scalar.activation(
                    sq[:], vts[t][:], AF.Square, accum_out=pm_acc[:, t : t + 1]
                )
                raw = scr_pool.tile([128, VFREE], F32, name="raw", tag="raw")
                nc.vector.tensor_scalar(
                    raw[:], vts[t][:], 1.0, None, ALU.mult, ALU.add,
                    accum_out=pm_acc[:, NVT + t : NVT + t + 1],
                )

            ps_pm = ps_pool.tile([ROWS_PER_CORE, 2 * NVT], F32, name="ps_pm")
            nc.tensor.matmul(ps_pm[:], pair_sb[:], pm_acc[:], start=True, stop=True)

            d2 = small_pool.tile([ROWS_PER_CORE, 2], F32, name="d2")
            nc.vector.tensor_reduce(
                d2[:, 0:1], ps_pm[:, 0:NVT], mybir.AxisListType.X, ALU.add
            )
            nc.vector.tensor_reduce(
                d2[:, 1:2], ps_pm[:, NVT : 2 * NVT], mybir.AxisListType.X, ALU.add
            )
            nc.sync.dma_start(out_pm[:], d2[:])

    nc.compile()
    return nc


def build_kernel_b():
    """Main kernel: derive vectors, build C^T, two matmul chains, write slab."""
    nc = bacc.Bacc("TRN2", target_bir_lowering=False, debug=False, num_devices=NCORES)

    # pm_pp: per-partition layout, col kt   = p[q + 128*kt],
    #        col 4+kt = msum[q + 128*kt]    (q = partition)
    pm_pp = nc.dram_tensor("pm_pp", [128, 2 * KT], F32, kind="ExternalInput")
    # row layouts (ordered 0..511), each on a single partition
    p_row_in = nc.dram_tensor("p_row", [1, D], F32, kind="ExternalInput")
    ms_row_in = nc.dram_tensor("ms_row", [1, D], F32, kind="ExternalInput")
    efull = nc.dram_tensor("efull", [KT, 128, S], F32R, kind="ExternalInput")
    ecols = nc.dram_tensor("ecols", [KT, 128, COLS_PER_CORE], F32R, kind="ExternalInput")
    out_re = nc.dram_tensor("out_re", [S, COLS_PER_CORE], F32, kind="ExternalOutput")
    out_im = nc.dram_tensor("out_im", [S, COLS_PER_CORE], F32, kind="ExternalOutput")

    with tile.TileContext(nc) as tc:
        with (
            tc.tile_pool(name="epool", bufs=1) as e_pool,
            tc.tile_pool(name="small", bufs=1) as small_pool,
            tc.tile_pool(name="cbuild", bufs=2) as cb_pool,
            tc.tile_pool(name="ctp", bufs=1) as ct_pool,
            tc.tile_pool(name="tsb", bufs=1) as t_pool,
            tc.tile_pool(name="ost", bufs=4) as o_pool,
            tc.tile_pool(name="psA", bufs=4, space="PSUM") as psA,
            tc.tile_pool(name="psB", bufs=4, space="PSUM") as psB,
        ):
            # -------- input DMAs (small first, then E) ------------------------
            pp = small_pool.tile([128, 2 * KT], F32, name="pp")
            nc.sync.dma_start(pp[:], pm_pp[:])
            prow = small_pool.tile([1, D], F32, name="prow")
            nc.sync.dma_start(prow[:], p_row_in[:])
            msrow = small_pool.tile([1, D], F32, name="msrow")
            nc.sync.dma_start(msrow[:], ms_row_in[:])

            ec_sb = []
            for kt in range(KT):
                ect = e_pool.tile(
                    [128, COLS_PER_CORE], F32R, name=f"ec{kt}", tag=f"ec{kt}"
                )
                nc.sync.dma_start(ect[:], ecols[kt])
                ec_sb.append(ect)
            e_sb = []
            for kt in range(KT):
                et = e_pool.tile([128, S], F32R, name=f"e{kt}", tag=f"e{kt}")
                nc.sync.dma_start(et[:], efull[kt])
                e_sb.append(et)

            # -------- PE warm-up during the small-vector derivation -----------
            ones_sb = small_pool.tile([1, 128], F32, name="ones_sb")
            nc.vector.memset(ones_sb[:], 1.0)
            warm_b = small_pool.tile([128, 128], BF16, name="warm_b")
            nc.gpsimd.memset(warm_b[:], 0.001)
            ps_w = psA.tile([128, 512], F32, name="ps_w", tag="t")
            for i in range(WARMUP_MMS):
                nc.tensor.matmul(
                    ps_w[:, 0:128], warm_b[:], warm_b[:],
                    start=(i == 0), stop=(i == WARMUP_MMS - 1),
                )

            # -------- derived vectors -----------------------------------------
            # per-partition [128, 4] each
            sp4 = small_pool.tile([128, KT], F32, name="sp4")
            a4 = small_pool.tile([128, KT], F32, name="a4")
            m24 = small_pool.tile([128, KT], F32, name="m24")
            nc.scalar.activation(sp4[:], pp[:, 0:KT], AF.Sqrt)
            nc.vector.scalar_tensor_tensor(
                a4[:], pp[:, KT : 2 * KT], INV_V, sp4[:], op0=ALU.mult, op1=ALU.mult
            )
            nc.vector.scalar_tensor_tensor(
                m24[:], pp[:, KT : 2 * KT], INV_V * INV_V, pp[:, KT : 2 * KT],
                op0=ALU.mult, op1=ALU.mult,
            )
            # row layout [1, 512] each
            sp_row = small_pool.tile([1, D], F32, name="sp_row")
            a_row = small_pool.tile([1, D], F32, name="a_row")
            m2_row = small_pool.tile([1, D], F32, name="m2_row")
            nc.scalar.activation(sp_row[:], prow[:], AF.Sqrt)
            nc.vector.scalar_tensor_tensor(
                a_row[:], msrow[:], INV_V, sp_row[:], op0=ALU.mult, op1=ALU.mult
            )
            nc.vector.scalar_tensor_tensor(
                m2_row[:], msrow[:], INV_V * INV_V, msrow[:],
                op0=ALU.mult, op1=ALU.mult,
            )

            # broadcast rows to 128 partitions via K=1 matmuls
            sp_bc = small_pool.tile([128, D], F32, name="sp_bc")
            a_bc = small_pool.tile([128, D], F32, name="a_bc")
            m2_bc = small_pool.tile([128, D], F32, name="m2_bc")
            for row, bc in ((sp_row, sp_bc), (a_row, a_bc), (m2_row, m2_bc)):
                ps_bc = psB.tile([128, D], F32, name="ps_bc", tag="o")
                nc.tensor.matmul(ps_bc[:], ones_sb[:], row[:], start=True, stop=True)
                nc.scalar.copy(bc[:], ps_bc[:])

            # -------- build C^T (real and imag) -------------------------------
            ct_r, ct_i = [], []
            for jt in range(KT):
                h = cb_pool.tile([128, D], F32, name="h", tag="h")
                nc.scalar.activation(
                    h[:], m2_bc[:], AF.Sqrt, bias=m24[:, jt : jt + 1], scale=1.0
                )
                rinv = cb_pool.tile([128, D], F32, name="rinv", tag="rinv")
                nc.vector.reciprocal_approx_fast(out=rinv[:], in_=h[:])
                rm = cb_pool.tile([128, D], F32, name="rm", tag="rm")
                nc.gpsimd.affine_select(
                    out=rm[:], in_=rinv[:],
                    pattern=[[-1, D]], compare_op=ALU.is_gt,
                    fill=0.0, base=128 * jt, channel_multiplier=1,
                )
                ctr = ct_pool.tile([128, D], F32R, name=f"ctr{jt}", tag=f"ctr{jt}")
                cti = ct_pool.tile([128, D], F32R, name=f"cti{jt}", tag=f"cti{jt}")
                nc.vector.scalar_tensor_tensor(
                    ctr[:], a_bc[:], sp4[:, jt : jt + 1], rm[:],
                    op0=ALU.mult, op1=ALU.mult,
                )
                nc.vector.scalar_tensor_tensor(
                    cti[:], sp_bc[:], a4[:, jt : jt + 1], rm[:],
                    op0=ALU.mult, op1=ALU.mult,
                )
                ct_r.append(ctr)
                ct_i.append(cti)

            # -------- T = C @ E[:, cols]  ([128, 512] = [T_r | T_i]) ----------
            t_sb = []
            for it in range(KT):
                ps_t = psA.tile([128, 2 * COLS_PER_CORE], F32, name="ps_t", tag="t")
                for jt in range(KT):
                    nc.tensor.matmul(
                        ps_t[:, 0:COLS_PER_CORE],
                        ct_r[jt][:, it * 128 : (it + 1) * 128],
                        ec_sb[jt][:],
                        start=(jt == 0), stop=(jt == KT - 1),
                    )
                for jt in range(KT):
                    nc.tensor.matmul(
                        ps_t[:, COLS_PER_CORE : 2 * COLS_PER_CORE],
                        ct_i[jt][:, it * 128 : (it + 1) * 128],
                        ec_sb[jt][:],
                        start=(jt == 0), stop=(jt == KT - 1),
                    )
                tsb = t_pool.tile(
                    [128, 2 * COLS_PER_CORE], F32R, name=f"tsb{it}", tag=f"tsb{it}"
                )
                if it % 2 == 0:
                    nc.scalar.copy(tsb[:], ps_t[:])
                else:
                    nc.vector.tensor_copy(tsb[:], ps_t[:])
                t_sb.append(tsb)

            # -------- out[:, cols] = E^T @ T ----------------------------------
            for st in range(MT):
                ps_o = psB.tile([128, 2 * COLS_PER_CORE], F32, name="ps_o", tag="o")
                for it in range(KT):
                    nc.tensor.matmul(
                        ps_o[:],
                        e_sb[it][:, st * 128 : (st + 1) * 128],
                        t_sb[it][:],
                        start=(it == 0), stop=(it == KT - 1),
                    )
                osb = o_pool.tile([128, 2 * COLS_PER_CORE], F32, name="osb", tag="osb")
                if st % 2 == 0:
                    nc.scalar.copy(osb[:], ps_o[:])
                else:
                    nc.vector.tensor_copy(osb[:], ps_o[:])
                nc.sync.dma_start(
                    out_re[st * 128 : (st + 1) * 128, :], osb[:, 0:COLS_PER_CORE]
                )
                nc.sync.dma_start(
                    out_im[st * 128 : (st + 1) * 128, :],
                    osb[:, COLS_PER_CORE : 2 * COLS_PER_CORE],
                )

    nc.compile()
    return nc


def _prepare_a_in_maps(vulns):
    vulns = np.ascontiguousarray(np.asarray(vulns, dtype=np.float32))
    pair = np.ascontiguousarray(
        np.repeat(np.eye(ROWS_PER_CORE, dtype=np.float32), 2, axis=0)
    )
    in_maps = []
    for c in range(NCORES):
        vsh = vulns[c * ROWS_PER_CORE : (c + 1) * ROWS_PER_CORE]
        in_maps.append(
            {
                "v128": np.ascontiguousarray(vsh.reshape(128, NVT, VFREE)),
                "pairmat": pair,
            }
        )
    return in_maps


def _prepare_b_in_maps(embed_table, domain_ids, p_full, msum_full):
    embed_table = np.ascontiguousarray(np.asarray(embed_table, dtype=np.float32))
    domain_ids = np.asarray(domain_ids).astype(np.int64)
    E = np.ascontiguousarray(embed_table[domain_ids])  # [512, 2048]
    e4 = _tf32_round(E).reshape(KT, 128, S)
    # per-partition layout [128, 8]
    pm_pp = np.empty((128, 2 * KT), dtype=np.float32)
    pm_pp[:, 0:KT] = p_full.reshape(KT, 128).T
    pm_pp[:, KT : 2 * KT] = msum_full.reshape(KT, 128).T
    p_row = np.ascontiguousarray(p_full.astype(np.float32).reshape(1, D))
    ms_row = np.ascontiguousarray(msum_full.astype(np.float32).reshape(1, D))
    in_maps = []
    for c in range(NCORES):
        in_maps.append(
            {
                "pm_pp": pm_pp,
                "p_row": p_row,
                "ms_row": ms_row,
                "efull": e4,
                "ecols": np.ascontiguousarray(
                    e4[:, :, c * COLS_PER_CORE : (c + 1) * COLS_PER_CORE]
                ),
            }
        )
    return in_maps


def kernel(vulns, embed_table, domain_ids, _trace=False):
    if "nc_a" not in _CACHE:
        _CACHE["nc_a"] = build_kernel_a()
    if "nc_b" not in _CACHE:
        _CACHE["nc_b"] = build_kernel_b()

    res_a = run_bass_kernel_spmd(
        _CACHE["nc_a"], _prepare_a_in_maps(vulns),
        core_ids=list(range(NCORES)), trace=_trace,
    )
    _CACHE["res_a"] = res_a
    p_full = np.concatenate([res_a.results[c]["out_pm"][:, 0] for c in range(NCORES)])
    msum_full = np.concatenate(
        [res_a.results[c]["out_pm"][:, 1] for c in range(NCORES)]
    )

    res_b = run_bass_kernel_spmd(
        _CACHE["nc_b"], _prepare_b_in_maps(embed_table, domain_ids, p_full, msum_full),
        core_ids=list(range(NCORES)), trace=_trace,
    )
    _CACHE["res_b"] = res_b

    out = np.empty((S, S), dtype=np.complex64)
    for c in range(NCORES):
        r = res_b.results[c]
        sl = slice(c * COLS_PER_CORE, (c + 1) * COLS_PER_CORE)
        out[:, sl] = r["out_re"] + 1j * r["out_im"]
    return out


if __name__ == "__main__":
    rng = np.random.default_rng(0)
    v = rng.standard_normal((D, V), dtype=np.float32)
    et = rng.standard_normal((D, S), dtype=np.float32)
    ids = np.arange(D, dtype=np.int32)
    out = kernel(v, et, ids)
    print(out.shape, out.dtype)
